# revision 24
# baseline (speedup 1.0000x reference)
"""Distributed Trainium2 kernel for EnhancedSelfAttention (causal attention
with additive ALiBi |i-j| bias) on 8 NeuronCores.

Math: for queries i and keys j<=i the bias is slope*(i-j), so
softmax_j(S_ij + slope*(i-j)) == softmax_j(S_ij - slope*j) — the slope*i term
is constant per row and cancels. Folding w_j = exp(-slope*j) into V's rows
(plus an appended w column for the denominator) turns the whole softmax into
exp(S) followed by a single PV matmul and a divide. w_j rounds to exactly 0
in fp16 beyond j ~ 18/slope, so most heads need only a few key blocks.

Sharding: 8 cores = 2 batches x 4 head groups (4 heads each). Each core
computes its partial projection output; partials are summed on the host.
All cores run one SPMD program: per-slot key-block budgets KBUD with head
4*(3-s)+g of group g in slot s, so budgets follow the slope bands.

Schedule: xt streams in t-major so QKV matmuls start after ~1.5MB; the
attention strips run query-chunk-major so each chunk's divides and partial
projection (and its output DMA) overlap the next chunk's strips; divides
are split (den-row copy on ACT early, broadcast/recip/mul two strips later)
so the PE never waits on a cross-engine dependency.
"""

import sys
import types

import numpy as np

import concourse.bass as bass
import concourse.mybir as mybir
import concourse.tile as tile
from concourse import bacc
from concourse.bass_utils import run_bass_kernel_spmd


def _ensure_axon_hooks():
    """concourse's trace path imports antenv.axon_hooks, which this image
    lacks; give it a no-op fallback so BASS_TRACE=1 can't crash the run."""
    try:
        import antenv.axon_hooks  # noqa: F401
    except Exception:
        try:
            import antenv
            mod = types.ModuleType("antenv.axon_hooks")
            mod.get_axon_ntff_profile_hook = lambda: None
            mod.set_axon_ntff_profile_hook = lambda h: None
            sys.modules["antenv.axon_hooks"] = mod
            antenv.axon_hooks = mod
        except Exception:
            pass


_ensure_axon_hooks()

F32 = mybir.dt.float32
F16 = mybir.dt.float16
ExpF = mybir.ActivationFunctionType.Exp

B, T, C = 2, 2048, 1024
NH, D = 16, 64
P = 128
NT = T // P            # 16 t tiles
KC = C // P            # 8 contraction subtiles for qkv/proj
QCH = 4                # q chunks of 512
KBUD = (16, 9, 3, 1)   # per-slot key-block budgets (fp16-underflow cutoffs)
                       # NOTE: (16,7,2,1) is numerically fine (~1e-4 tail) but
                       # measured SLOWER: 1-strip slots churn the psO ring and
                       # the resulting PE hiccups drop the tensor p-state.
N_CORES = 8

# head -> (group, slot): slot s of group g gets head 4*(3-s)+g, so budgets
# follow slope bands: slot0 h12..15 (16 blocks), slot1 h8..11 (9), slot2
# h4..7 (3), slot3 h0..3 (1).  Beyond each budget exp(-slope*j) rounds to
# zero in fp16, so vv rows there are exactly 0 and contribute nothing.
GROUP_HEADS = [(12 + g, 8 + g, 4 + g, g) for g in range(4)]


def _nslots(mt):
    """Slots that still need key tile mt (budget > mt)."""
    return sum(1 for b in KBUD if b > mt)


TRACE = False  # test harness sets kernel.TRACE = True for NTFF profiling

_CACHE = {}


def _slopes():
    i = np.arange(1, NH + 1, dtype=np.float64)
    return (1.0 / np.power(2.0, 8.0 * i / NH)).astype(np.float64)


def _build_program():
    nc = bacc.Bacc("TRN2", target_bir_lowering=False, debug=False,
                   num_devices=N_CORES)

    xt_d = nc.dram_tensor("xt", [C, T], F16, kind="ExternalInput").ap()
    wq_d = nc.dram_tensor("wq", [C, 4 * D], F16, kind="ExternalInput").ap()
    wk_d = nc.dram_tensor("wk", [C, 4 * D], F16, kind="ExternalInput").ap()
    wv_d = nc.dram_tensor("wv", [C, 4 * D], F16, kind="ExternalInput").ap()
    wp_d = nc.dram_tensor("wp", [4 * D, C], F16, kind="ExternalInput").ap()
    wcol_d = nc.dram_tensor("wcol", [T, 4], F32, kind="ExternalInput").ap()
    masks_d = nc.dram_tensor("masks", [P, 4 * 512], F16, kind="ExternalInput").ap()
    y_d = nc.dram_tensor("y", [T, C], F16, kind="ExternalOutput").ap()

    with tile.TileContext(nc) as tc:
        with (
            nc.allow_low_precision(reason="fp16 matmul operands by design"),
            tc.tile_pool(name="const", bufs=1) as const,
            tc.tile_pool(name="psB", bufs=2, space="PSUM") as psB,
            tc.tile_pool(name="psO", bufs=4, space="PSUM") as psO,
            tc.tile_pool(name="pp", bufs=4) as pp,
            tc.tile_pool(name="rr", bufs=3) as rr,
            tc.tile_pool(name="rbp", bufs=3) as rbp,
            tc.tile_pool(name="yp", bufs=6) as yp,
        ):
            # ---- persistent SBUF loads, ordered by first use.
            # One big 3D-AP DMA per tensor: descriptors (>=512B each) fan out
            # across all 16 DMA engines, and the sync sequencer only pays its
            # ~600ns trigger cost ~10x instead of ~60x.
            # xt arrives t-chunk-major (all 8 k-subtiles of a 512-col chunk
            # together) so the first QKV groups start after ~1.5MB.
            wq_sb = const.tile([P, KC, 4 * D], F16, tag="wq")
            wk_sb = const.tile([P, KC, 4 * D], F16, tag="wk")
            wv_sb = const.tile([P, KC, 4 * D], F16, tag="wv")
            xt_sb = const.tile([P, KC, T], F16, tag="xt")        # 32KB/part

            # weights/masks/wp triggered from the (idle) scalar engine's
            # HWDGE queue so they don't serialize behind sync's xt triggers
            def w_dma(w_sb, w_d):
                nc.scalar.dma_start(
                    w_sb[:], w_d.rearrange("(k p) c -> p k c", p=P))

            def xt_dma(tch):
                nc.sync.dma_start(
                    xt_sb[:, :, tch * 512:(tch + 1) * 512],
                    xt_d.rearrange("(k p) t -> p k t", p=P)
                    [:, :, tch * 512:(tch + 1) * 512])

            def xt_dma_half(tch, kh):
                nc.sync.dma_start(
                    xt_sb[:, kh * 4:(kh + 1) * 4, tch * 512:(tch + 1) * 512],
                    xt_d.rearrange("(k p) t -> p k t", p=P)
                    [:, kh * 4:(kh + 1) * 4, tch * 512:(tch + 1) * 512])

            w_dma(wq_sb, wq_d)
            xt_dma_half(0, 0)      # k-halves: first QKV matmuls start sooner
            xt_dma_half(0, 1)
            w_dma(wv_sb, wv_d)
            wcol_sb = const.tile([P, NT, 4], F32, tag="wcol")
            nc.sync.dma_start(
                wcol_sb[:], wcol_d.rearrange("(n p) c -> p n c", p=P))
            w_dma(wk_sb, wk_d)
            xt_dma_half(1, 0)
            xt_dma_half(1, 1)
            for tch in range(2, 4):
                xt_dma(tch)
            masks_sb = const.tile([P, 4 * 512], F16, tag="masks")
            nc.scalar.dma_start(masks_sb[:], masks_d[:])
            wp_sb = const.tile([P, 2, C], F16, tag="wp")
            nc.scalar.dma_start(
                wp_sb[:], wp_d.rearrange("(j p) c -> p j c", p=P))
            ones_sb = const.tile([1, D], F16, tag="ones")
            nc.any.memset(ones_sb[:], 1.0)
            # warm the ACT exp table during the DMA wait
            warm_sb = const.tile([1, D], F16, tag="warm")
            nc.scalar.activation(warm_sb[:], ones_sb[:], ExpF)

            qt_sb = [const.tile([P, T], F16, tag=f"qt{m}", name=f"qt{m}")
                     for m in range(2)]
            kt_sb = [const.tile([P, T], F16, tag=f"kt{m}", name=f"kt{m}")
                     for m in range(2)]
            vv_sb = const.tile([P, NT, 4, 65], F16, tag="vv")
            ot_sb = [const.tile([P, T], F16, tag=f"ot{m}", name=f"ot{m}")
                     for m in range(2)]

            # ---- phase 1: QT/KT ([d, t] layout) interleaved with V groups,
            # sequenced to match t-chunk DMA arrival.
            def emit_qkt_group(i):
                w_sb, dst = ((wq_sb, qt_sb), (wk_sb, kt_sb))[i // 8]
                m, nch = divmod(i % 8, QCH)
                ps = psB.tile([P, 1024], F32, tag="mm", name="ps_qkt")
                for k in range(KC):
                    nc.tensor.matmul(
                        ps[:, 0:512],
                        w_sb[:, k, m * P:(m + 1) * P],
                        xt_sb[:, k, nch * 512:(nch + 1) * 512],
                        start=(k == 0), stop=(k == KC - 1))
                nc.vector.tensor_copy(
                    dst[m][:, nch * 512:(nch + 1) * 512], ps[:, 0:512])

            def emit_v_group(mt):
                ns = _nslots(mt)   # only slots with budget > mt read vv[mt]
                psv = psB.tile([P, 1024], F32, tag="mm", name="ps_v")
                for k in range(KC):
                    nc.tensor.matmul(
                        psv[:, 0:ns * D],
                        xt_sb[:, k, mt * P:(mt + 1) * P],
                        wv_sb[:, k, 0:ns * D],
                        start=(k == 0), stop=(k == KC - 1))
                # evictions alternate DVE/ACT per mt so neither engine's
                # backlog stalls the psB ring during phase 1
                for s in range(ns):
                    if mt % 2 == 0:
                        nc.vector.tensor_scalar_mul(
                            vv_sb[:, mt, s, 0:D], psv[:, s * D:(s + 1) * D],
                            wcol_sb[:, mt, s: s + 1])
                    else:
                        nc.scalar.activation(
                            vv_sb[:, mt, s, 0:D], psv[:, s * D:(s + 1) * D],
                            mybir.ActivationFunctionType.Copy,
                            scale=wcol_sb[:, mt, s: s + 1])

            # q m0 nch j = j; q m1 = 4+j; k m0 = 8+j; k m1 nch0 = 12
            # (kt m1 nch1..3 never read: slots 2/3 use keys < 512).
            phase1 = [0, "v0", "v1", "v2", "v3", 8,
                      1, "v4", "v5", "v6", "v7", 9,
                      2, "v8", "v9", "v10", "v11", 10,
                      3, "v12", "v13", "v14", "v15", 11,
                      4, 12, 5, 6, 7]
            for it in phase1:
                if isinstance(it, str):
                    emit_v_group(int(it[1:]))
                else:
                    emit_qkt_group(it)
                if it == "v0":
                    # den columns for all (t, slot) in one strided copy
                    nc.vector.tensor_copy(vv_sb[:, :, :, 64], wcol_sb[:])

            # ---- phase 2: attention, query-chunk-major software pipeline.
            # Each strip = up to 2 key blocks of S^T for one (slot, q-chunk).
            # Issue order per step: S matmuls(i), exp/mask(i), PV(i-1).
            # When a (s,qc) finishes its PV, its den row is copied on ACT at
            # once; the broadcast/recip/mul run 2 strips later, and the
            # partial projection for a finished q-chunk streams out one
            # 512-col half-tile per strip.
            strips = []
            for qc in range(QCH):
                for s in range(4):
                    kmax = min(KBUD[s], 4 * qc + 4)
                    for g in range((kmax + 1) // 2):
                        kts = [kt for kt in (2 * g, 2 * g + 1) if kt < kmax]
                        strips.append((s, qc, g, kts, kmax))
            n_strips = len(strips)

            opsums = {}        # (s, qc) -> psum tile
            dens = {}          # (s, qc) -> dh sbuf tile
            pendings = []      # [(flush_at, strip, pst)] FIFO
            divides = []       # [(emit_at, (s, qc))]
            projq = []         # [(emit_at, mt, half)]

            def emit_pv(strip, pst):
                s, qc, g, kts, kmax = strip
                for d_, kt in enumerate(kts):
                    nc.tensor.matmul(
                        opsums[(s, qc)][0:65, :],
                        vv_sb[:, kt, s, :],
                        pst[:, d_ * 512:(d_ + 1) * 512],
                        start=(kt == 0), stop=(kt == kmax - 1))
                if kts[-1] == kmax - 1:
                    # accumulation done: den row to SBUF at once (ACT) so the
                    # later broadcast matmul never stalls the PE on this copy.
                    dh = rr.tile([1, 512], F16, tag="dh", name="dh")
                    nc.scalar.copy(dh[:], opsums[(s, qc)][64:65, :])
                    dens[(s, qc)] = dh

            def emit_divide(s, qc):
                opsum = opsums.pop((s, qc))
                dh = dens.pop((s, qc))
                ot_t = ot_sb[s // 2]
                base = (s % 2) * D
                rb = psO.tile([P, 512], F32, tag="o", name="rb")
                nc.tensor.matmul(rb[0:D, :], ones_sb[:], dh[:],
                                 start=True, stop=True)
                rbs = rbp.tile([D, 512], F32, tag="rbs", name="rbs")
                nc.vector.reciprocal_approx_fast(rbs[:], rb[0:D, :])
                nc.vector.tensor_mul(
                    ot_t[base:base + D, qc * 512:(qc + 1) * 512],
                    opsum[0:64, :], rbs[:])

            yts = {}           # mt -> yt tile (DMA'd once both halves done)

            def emit_proj_half(mt, h):
                ps = psO.tile([P, 512], F32, tag="o", name="ps_proj")
                for j in range(2):
                    nc.tensor.matmul(
                        ps[:], ot_sb[j][:, mt * P:(mt + 1) * P],
                        wp_sb[:, j, h * 512:(h + 1) * 512],
                        start=(j == 0), stop=(j == 1))
                if h == 0:
                    yts[mt] = yp.tile([P, 1024], F16, tag="y", name="yt")
                    nc.scalar.copy(yts[mt][:, 0:512], ps[:])
                else:
                    yt = yts.pop(mt)
                    nc.vector.tensor_copy(yt[:, 512:1024], ps[:])
                    nc.sync.dma_start(y_d[mt * P:(mt + 1) * P, :], yt[:])

            def flush_pv(i):
                while pendings and pendings[0][0] <= i:
                    _, pstrip, ppst = pendings.pop(0)
                    emit_pv(pstrip, ppst)
                    if pstrip[3][-1] == pstrip[4] - 1:  # stop chunk of (s,qc)
                        divides.append((i + 2, (pstrip[0], pstrip[1])))

            def drain(i):
                while divides and divides[0][0] <= i:
                    _, (ds, dqc) = divides.pop(0)
                    emit_divide(ds, dqc)
                    if ds == 3:  # last slot of dqc -> queue its projection
                        for n_, (mt, h) in enumerate(
                                (m_, h_) for m_ in range(4 * dqc, 4 * dqc + 4)
                                for h_ in range(2)):
                            projq.append((i + 2 + n_, mt, h))
                if projq and projq[0][0] <= i:
                    _, mt, h = projq.pop(0)
                    emit_proj_half(mt, h)

            for i, strip in enumerate(strips):
                s, qc, g, kts, kmax = strip
                drain(i)
                if (s, qc) not in opsums:
                    opsums[(s, qc)] = psO.tile([P, 512], F32, tag="o",
                                               name="opsum")
                qt_t = qt_sb[s // 2]
                kt_t = kt_sb[s // 2]
                base = (s % 2) * D
                w = len(kts)
                sps = psB.tile([P, 1024], F32, tag="mm", name="sps")
                for d_, kt in enumerate(kts):
                    nc.tensor.matmul(
                        sps[:, d_ * 512:(d_ + 1) * 512],
                        kt_t[base:base + D, kt * P:(kt + 1) * P],
                        qt_t[base:base + D, qc * 512:(qc + 1) * 512],
                        start=True, stop=True)
                pst = pp.tile([P, 1024], F16, tag="p", name="pst")
                nc.scalar.activation(pst[:, 0:512 * w], sps[:, 0:512 * w], ExpF)
                masked = g in (2 * qc, 2 * qc + 1)
                if g == 2 * qc:  # diagonal blocks delta 0,1
                    nc.vector.tensor_mul(pst[:, 0:512 * w], pst[:, 0:512 * w],
                                         masks_sb[:, 0:512 * w])
                elif g == 2 * qc + 1:  # diagonal blocks delta 2,3
                    nc.vector.tensor_mul(pst[:, 0:512 * w], pst[:, 0:512 * w],
                                         masks_sb[:, 1024:1024 + 512 * w])
                flush_pv(i)
                # masked strips' PV lags 2 so the PE never waits on the
                # exp->mask cross-engine chain; PV order per (s,qc) is
                # preserved (the g==2qc strip then flushes with g==2qc+1).
                pendings.append((i + (2 if masked else 1), strip, pst))

            # ---- tail: final PVs, remaining divides and projections.
            i = n_strips
            while pendings or divides or projq:
                flush_pv(i)
                drain(i)
                i += 1

    nc.compile()
    return nc


def _host_prep(x, w_qkv, w_proj):
    """Per-core input maps."""
    slopes = _slopes()
    scale = 1.0 / np.sqrt(D)
    in_maps = []
    xt_by_b = [np.ascontiguousarray(x[b].T).astype(np.float16) for b in range(B)]

    # masks: delta in 0..3, [128, 512] each: valid iff r <= c - 128*delta
    rr_ = np.arange(P)[:, None]
    cc = np.arange(512)[None, :]
    masks = np.concatenate(
        [(rr_ <= cc - P * d).astype(np.float16) for d in range(4)], axis=1)

    group_data = []
    for g in range(4):
        H = GROUP_HEADS[g]
        cols = np.concatenate([np.arange(h * D, (h + 1) * D) for h in H])
        wq = (w_qkv[:, cols] * scale).astype(np.float16)
        wk = w_qkv[:, C + cols].astype(np.float16)
        wv = w_qkv[:, 2 * C + cols].astype(np.float16)
        wp = np.ascontiguousarray(w_proj[cols, :]).astype(np.float16)
        t = np.arange(T, dtype=np.float64)
        wcol = np.stack(
            [np.exp(-slopes[h] * t) for h in H], axis=1).astype(np.float32)
        group_data.append((wq, wk, wv, wp, wcol))

    for c in range(N_CORES):
        b, g = divmod(c, 4)
        wq, wk, wv, wp, wcol = group_data[g]
        in_maps.append({
            "xt": xt_by_b[b], "wq": wq, "wk": wk, "wv": wv, "wp": wp,
            "wcol": wcol, "masks": masks,
        })
    return in_maps


def kernel(x, w_qkv, w_proj):
    if "nc" not in _CACHE:
        _CACHE["nc"] = _build_program()
    nc = _CACHE["nc"]

    in_maps = _host_prep(np.asarray(x, np.float32), np.asarray(w_qkv, np.float32),
                         np.asarray(w_proj, np.float32))
    res = run_bass_kernel_spmd(nc, in_maps, list(range(N_CORES)), trace=TRACE)
    _CACHE["last_result"] = res

    y = np.zeros((B, T, C), dtype=np.float64)
    for c in range(N_CORES):
        b = c // 4
        y[b] += res.results[c]["y"].astype(np.float64)
    return y.astype(np.float32)


# revision 25
# speedup vs baseline: 1.0501x; 1.0501x over previous
"""Distributed Trainium2 kernel for EnhancedSelfAttention (causal attention
with additive ALiBi |i-j| bias) on 8 NeuronCores.

Math: for queries i and keys j<=i the bias is slope*(i-j), so
softmax_j(S_ij + slope*(i-j)) == softmax_j(S_ij - slope*j) — the slope*i term
is constant per row and cancels. Folding w_j = exp(-slope*j) into V's rows
(plus an appended w column for the denominator) turns the whole softmax into
exp(S) followed by a single PV matmul and a divide. w_j rounds to exactly 0
in fp16 beyond j ~ 18/slope, so most heads need only a few key blocks.

Sharding: 8 cores = 2 batches x 4 head groups (4 heads each). Each core
computes its partial projection output; partials are summed on the host.
All cores run one SPMD program: per-slot key-block budgets KBUD with head
4*(3-s)+g of group g in slot s, so budgets follow the slope bands.

Schedule: xt streams in t-major so QKV matmuls start after ~1.5MB; the
attention strips run query-chunk-major so each chunk's divides and partial
projection (and its output DMA) overlap the next chunk's strips; divides
are split (den-row copy on ACT early, broadcast/recip/mul two strips later)
so the PE never waits on a cross-engine dependency.
"""

import sys
import types

import numpy as np

import concourse.bass as bass
import concourse.mybir as mybir
import concourse.tile as tile
from concourse import bacc
from concourse.bass_utils import run_bass_kernel_spmd


def _ensure_axon_hooks():
    """concourse's trace path imports antenv.axon_hooks, which this image
    lacks; give it a no-op fallback so BASS_TRACE=1 can't crash the run."""
    try:
        import antenv.axon_hooks  # noqa: F401
    except Exception:
        try:
            import antenv
            mod = types.ModuleType("antenv.axon_hooks")
            mod.get_axon_ntff_profile_hook = lambda: None
            mod.set_axon_ntff_profile_hook = lambda h: None
            sys.modules["antenv.axon_hooks"] = mod
            antenv.axon_hooks = mod
        except Exception:
            pass


_ensure_axon_hooks()

F32 = mybir.dt.float32
F16 = mybir.dt.float16
ExpF = mybir.ActivationFunctionType.Exp

B, T, C = 2, 2048, 1024
NH, D = 16, 64
P = 128
NT = T // P            # 16 t tiles
KC = C // P            # 8 contraction subtiles for qkv/proj
QCH = 4                # q chunks of 512
KBUD = (16, 9, 3, 1)   # per-slot key-block budgets (fp16-underflow cutoffs)
                       # NOTE: (16,7,2,1) is numerically fine (~1e-4 tail) but
                       # measured SLOWER: 1-strip slots churn the psO ring and
                       # the resulting PE hiccups drop the tensor p-state.
N_CORES = 8

# head -> (group, slot): slot s of group g gets head 4*(3-s)+g, so budgets
# follow slope bands: slot0 h12..15 (16 blocks), slot1 h8..11 (9), slot2
# h4..7 (3), slot3 h0..3 (1).  Beyond each budget exp(-slope*j) rounds to
# zero in fp16, so vv rows there are exactly 0 and contribute nothing.
GROUP_HEADS = [(12 + g, 8 + g, 4 + g, g) for g in range(4)]


def _nslots(mt):
    """Slots that still need key tile mt (budget > mt)."""
    return sum(1 for b in KBUD if b > mt)


TRACE = False  # test harness sets kernel.TRACE = True for NTFF profiling

_CACHE = {}


def _slopes():
    i = np.arange(1, NH + 1, dtype=np.float64)
    return (1.0 / np.power(2.0, 8.0 * i / NH)).astype(np.float64)


def _build_program():
    nc = bacc.Bacc("TRN2", target_bir_lowering=False, debug=False,
                   num_devices=N_CORES)

    xt_d = nc.dram_tensor("xt", [C, T], F16, kind="ExternalInput").ap()
    wq_d = nc.dram_tensor("wq", [C, 4 * D], F16, kind="ExternalInput").ap()
    wk_d = nc.dram_tensor("wk", [C, 4 * D], F16, kind="ExternalInput").ap()
    wv_d = nc.dram_tensor("wv", [C, 4 * D], F16, kind="ExternalInput").ap()
    wp_d = nc.dram_tensor("wp", [4 * D, C], F16, kind="ExternalInput").ap()
    wcol_d = nc.dram_tensor("wcol", [T, 4], F32, kind="ExternalInput").ap()
    masks_d = nc.dram_tensor("masks", [P, 4 * 512], F16, kind="ExternalInput").ap()
    y_d = nc.dram_tensor("y", [T, C], F16, kind="ExternalOutput").ap()

    with tile.TileContext(nc) as tc:
        with (
            nc.allow_low_precision(reason="fp16 matmul operands by design"),
            tc.tile_pool(name="const", bufs=1) as const,
            tc.tile_pool(name="psB", bufs=2, space="PSUM") as psB,
            tc.tile_pool(name="psO", bufs=4, space="PSUM") as psO,
            tc.tile_pool(name="pp", bufs=4) as pp,
            tc.tile_pool(name="rr", bufs=3) as rr,
            tc.tile_pool(name="rbp", bufs=3) as rbp,
            tc.tile_pool(name="yp", bufs=6) as yp,
        ):
            # ---- persistent SBUF loads, ordered by first use.
            # One big 3D-AP DMA per tensor: descriptors (>=512B each) fan out
            # across all 16 DMA engines, and the sync sequencer only pays its
            # ~600ns trigger cost ~10x instead of ~60x.
            # xt arrives t-chunk-major (all 8 k-subtiles of a 512-col chunk
            # together) so the first QKV groups start after ~1.5MB.
            wq_sb = const.tile([P, KC, 4 * D], F16, tag="wq")
            wk_sb = const.tile([P, KC, 4 * D], F16, tag="wk")
            wv_sb = const.tile([P, KC, 4 * D], F16, tag="wv")
            xt_sb = const.tile([P, KC, T], F16, tag="xt")        # 32KB/part

            def w_dma(w_sb, w_d):
                nc.sync.dma_start(
                    w_sb[:], w_d.rearrange("(k p) c -> p k c", p=P))

            def xt_dma(tch):
                nc.sync.dma_start(
                    xt_sb[:, :, tch * 512:(tch + 1) * 512],
                    xt_d.rearrange("(k p) t -> p k t", p=P)
                    [:, :, tch * 512:(tch + 1) * 512])

            def xt_dma_half(tch, kh):
                nc.sync.dma_start(
                    xt_sb[:, kh * 4:(kh + 1) * 4, tch * 512:(tch + 1) * 512],
                    xt_d.rearrange("(k p) t -> p k t", p=P)
                    [:, kh * 4:(kh + 1) * 4, tch * 512:(tch + 1) * 512])

            w_dma(wq_sb, wq_d)
            xt_dma_half(0, 0)      # k-halves: first QKV matmuls start sooner
            xt_dma_half(0, 1)
            w_dma(wv_sb, wv_d)
            wcol_sb = const.tile([P, NT, 4], F32, tag="wcol")
            nc.sync.dma_start(
                wcol_sb[:], wcol_d.rearrange("(n p) c -> p n c", p=P))
            w_dma(wk_sb, wk_d)
            xt_dma_half(1, 0)
            xt_dma_half(1, 1)
            for tch in range(2, 4):
                xt_dma(tch)
            masks_sb = const.tile([P, 4 * 512], F16, tag="masks")
            nc.sync.dma_start(masks_sb[:], masks_d[:])
            wp_sb = const.tile([P, 2, C], F16, tag="wp")
            nc.sync.dma_start(
                wp_sb[:], wp_d.rearrange("(j p) c -> p j c", p=P))
            ones_sb = const.tile([1, D], F16, tag="ones")
            nc.any.memset(ones_sb[:], 1.0)
            # warm the ACT exp table during the DMA wait
            warm_sb = const.tile([1, D], F16, tag="warm")
            nc.scalar.activation(warm_sb[:], ones_sb[:], ExpF)

            qt_sb = [const.tile([P, T], F16, tag=f"qt{m}", name=f"qt{m}")
                     for m in range(2)]
            kt_sb = [const.tile([P, T], F16, tag=f"kt{m}", name=f"kt{m}")
                     for m in range(2)]
            vv_sb = const.tile([P, NT, 4, 65], F16, tag="vv")
            ot_sb = [const.tile([P, T], F16, tag=f"ot{m}", name=f"ot{m}")
                     for m in range(2)]

            # ---- phase 1: QT/KT ([d, t] layout) interleaved with V groups,
            # sequenced to match t-chunk DMA arrival.
            def emit_qkt_group(i):
                w_sb, dst = ((wq_sb, qt_sb), (wk_sb, kt_sb))[i // 8]
                m, nch = divmod(i % 8, QCH)
                ps = psB.tile([P, 1024], F32, tag="mm", name="ps_qkt")
                for k in range(KC):
                    nc.tensor.matmul(
                        ps[:, 0:512],
                        w_sb[:, k, m * P:(m + 1) * P],
                        xt_sb[:, k, nch * 512:(nch + 1) * 512],
                        start=(k == 0), stop=(k == KC - 1))
                nc.vector.tensor_copy(
                    dst[m][:, nch * 512:(nch + 1) * 512], ps[:, 0:512])

            def emit_v_group(mt):
                ns = _nslots(mt)   # only slots with budget > mt read vv[mt]
                psv = psB.tile([P, 1024], F32, tag="mm", name="ps_v")
                for k in range(KC):
                    nc.tensor.matmul(
                        psv[:, 0:ns * D],
                        xt_sb[:, k, mt * P:(mt + 1) * P],
                        wv_sb[:, k, 0:ns * D],
                        start=(k == 0), stop=(k == KC - 1))
                for s in range(ns):
                    nc.vector.tensor_scalar_mul(
                        vv_sb[:, mt, s, 0:D], psv[:, s * D:(s + 1) * D],
                        wcol_sb[:, mt, s: s + 1])

            # q m0 nch j = j; q m1 = 4+j; k m0 = 8+j; k m1 nch0 = 12
            # (kt m1 nch1..3 never read: slots 2/3 use keys < 512).
            phase1 = [0, "v0", "v1", "v2", "v3", 8,
                      1, "v4", "v5", "v6", "v7", 9,
                      2, "v8", "v9", "v10", "v11", 10,
                      3, "v12", "v13", "v14", "v15", 11,
                      4, 12, 5, 6, 7]
            for it in phase1:
                if isinstance(it, str):
                    emit_v_group(int(it[1:]))
                else:
                    emit_qkt_group(it)
                if it == "v0":
                    # den columns for all (t, slot) in one strided copy
                    nc.vector.tensor_copy(vv_sb[:, :, :, 64], wcol_sb[:])

            # ---- phase 2: attention, query-chunk-major software pipeline.
            # Each strip = up to 2 key blocks of S^T for one (slot, q-chunk).
            # Issue order per step: S matmuls(i), exp/mask(i), PV(i-1).
            # When a (s,qc) finishes its PV, its den row is copied on ACT at
            # once; the broadcast/recip/mul run 2 strips later, and the
            # partial projection for a finished q-chunk streams out one
            # 512-col half-tile per strip.
            strips = []
            for qc in range(QCH):
                for s in range(4):
                    kmax = min(KBUD[s], 4 * qc + 4)
                    for g in range((kmax + 1) // 2):
                        kts = [kt for kt in (2 * g, 2 * g + 1) if kt < kmax]
                        strips.append((s, qc, g, kts, kmax))
            n_strips = len(strips)

            opsums = {}        # (s, qc) -> psum tile
            dens = {}          # (s, qc) -> dh sbuf tile
            pendings = []      # [(flush_at, strip, pst)] FIFO
            divides = []       # [(emit_at, (s, qc))]
            projq = []         # [(emit_at, mt, half)]

            def emit_pv(strip, pst):
                s, qc, g, kts, kmax = strip
                for d_, kt in enumerate(kts):
                    nc.tensor.matmul(
                        opsums[(s, qc)][0:65, :],
                        vv_sb[:, kt, s, :],
                        pst[:, d_ * 512:(d_ + 1) * 512],
                        start=(kt == 0), stop=(kt == kmax - 1))
                if kts[-1] == kmax - 1:
                    # accumulation done: den row to SBUF at once (ACT) so the
                    # later broadcast matmul never stalls the PE on this copy.
                    dh = rr.tile([1, 512], F16, tag="dh", name="dh")
                    nc.scalar.copy(dh[:], opsums[(s, qc)][64:65, :])
                    dens[(s, qc)] = dh

            def emit_divide(s, qc):
                opsum = opsums.pop((s, qc))
                dh = dens.pop((s, qc))
                ot_t = ot_sb[s // 2]
                base = (s % 2) * D
                rb = psO.tile([P, 512], F32, tag="o", name="rb")
                nc.tensor.matmul(rb[0:D, :], ones_sb[:], dh[:],
                                 start=True, stop=True)
                rbs = rbp.tile([D, 512], F32, tag="rbs", name="rbs")
                nc.vector.reciprocal_approx_fast(rbs[:], rb[0:D, :])
                nc.vector.tensor_mul(
                    ot_t[base:base + D, qc * 512:(qc + 1) * 512],
                    opsum[0:64, :], rbs[:])

            yts = {}           # mt -> yt tile (DMA'd once both halves done)

            def emit_proj_half(mt, h):
                ps = psO.tile([P, 512], F32, tag="o", name="ps_proj")
                for j in range(2):
                    nc.tensor.matmul(
                        ps[:], ot_sb[j][:, mt * P:(mt + 1) * P],
                        wp_sb[:, j, h * 512:(h + 1) * 512],
                        start=(j == 0), stop=(j == 1))
                if h == 0:
                    yts[mt] = yp.tile([P, 1024], F16, tag="y", name="yt")
                    nc.scalar.copy(yts[mt][:, 0:512], ps[:])
                else:
                    yt = yts.pop(mt)
                    nc.vector.tensor_copy(yt[:, 512:1024], ps[:])
                    nc.sync.dma_start(y_d[mt * P:(mt + 1) * P, :], yt[:])

            def flush_pv(i):
                while pendings and pendings[0][0] <= i:
                    _, pstrip, ppst = pendings.pop(0)
                    emit_pv(pstrip, ppst)
                    if pstrip[3][-1] == pstrip[4] - 1:  # stop chunk of (s,qc)
                        divides.append((i + 2, (pstrip[0], pstrip[1])))

            def drain(i):
                while divides and divides[0][0] <= i:
                    _, (ds, dqc) = divides.pop(0)
                    emit_divide(ds, dqc)
                    if ds == 3:  # last slot of dqc -> queue its projection
                        for n_, (mt, h) in enumerate(
                                (m_, h_) for m_ in range(4 * dqc, 4 * dqc + 4)
                                for h_ in range(2)):
                            projq.append((i + 2 + n_, mt, h))
                if projq and projq[0][0] <= i:
                    _, mt, h = projq.pop(0)
                    emit_proj_half(mt, h)

            for i, strip in enumerate(strips):
                s, qc, g, kts, kmax = strip
                drain(i)
                if (s, qc) not in opsums:
                    opsums[(s, qc)] = psO.tile([P, 512], F32, tag="o",
                                               name="opsum")
                qt_t = qt_sb[s // 2]
                kt_t = kt_sb[s // 2]
                base = (s % 2) * D
                w = len(kts)
                sps = psB.tile([P, 1024], F32, tag="mm", name="sps")
                for d_, kt in enumerate(kts):
                    nc.tensor.matmul(
                        sps[:, d_ * 512:(d_ + 1) * 512],
                        kt_t[base:base + D, kt * P:(kt + 1) * P],
                        qt_t[base:base + D, qc * 512:(qc + 1) * 512],
                        start=True, stop=True)
                pst = pp.tile([P, 1024], F16, tag="p", name="pst")
                nc.scalar.activation(pst[:, 0:512 * w], sps[:, 0:512 * w], ExpF)
                masked = g in (2 * qc, 2 * qc + 1)
                if g == 2 * qc:  # diagonal blocks delta 0,1
                    nc.vector.tensor_mul(pst[:, 0:512 * w], pst[:, 0:512 * w],
                                         masks_sb[:, 0:512 * w])
                elif g == 2 * qc + 1:  # diagonal blocks delta 2,3
                    nc.vector.tensor_mul(pst[:, 0:512 * w], pst[:, 0:512 * w],
                                         masks_sb[:, 1024:1024 + 512 * w])
                flush_pv(i)
                # masked strips' PV lags 2 so the PE never waits on the
                # exp->mask cross-engine chain; PV order per (s,qc) is
                # preserved (the g==2qc strip then flushes with g==2qc+1).
                pendings.append((i + (2 if masked else 1), strip, pst))

            # ---- tail: final PVs, remaining divides and projections.
            i = n_strips
            while pendings or divides or projq:
                flush_pv(i)
                drain(i)
                i += 1

    nc.compile()
    return nc


def _host_prep(x, w_qkv, w_proj):
    """Per-core input maps."""
    slopes = _slopes()
    scale = 1.0 / np.sqrt(D)
    in_maps = []
    xt_by_b = [np.ascontiguousarray(x[b].T).astype(np.float16) for b in range(B)]

    # masks: delta in 0..3, [128, 512] each: valid iff r <= c - 128*delta
    rr_ = np.arange(P)[:, None]
    cc = np.arange(512)[None, :]
    masks = np.concatenate(
        [(rr_ <= cc - P * d).astype(np.float16) for d in range(4)], axis=1)

    group_data = []
    for g in range(4):
        H = GROUP_HEADS[g]
        cols = np.concatenate([np.arange(h * D, (h + 1) * D) for h in H])
        wq = (w_qkv[:, cols] * scale).astype(np.float16)
        wk = w_qkv[:, C + cols].astype(np.float16)
        wv = w_qkv[:, 2 * C + cols].astype(np.float16)
        wp = np.ascontiguousarray(w_proj[cols, :]).astype(np.float16)
        t = np.arange(T, dtype=np.float64)
        wcol = np.stack(
            [np.exp(-slopes[h] * t) for h in H], axis=1).astype(np.float32)
        group_data.append((wq, wk, wv, wp, wcol))

    for c in range(N_CORES):
        b, g = divmod(c, 4)
        wq, wk, wv, wp, wcol = group_data[g]
        in_maps.append({
            "xt": xt_by_b[b], "wq": wq, "wk": wk, "wv": wv, "wp": wp,
            "wcol": wcol, "masks": masks,
        })
    return in_maps


def kernel(x, w_qkv, w_proj):
    if "nc" not in _CACHE:
        _CACHE["nc"] = _build_program()
    nc = _CACHE["nc"]

    in_maps = _host_prep(np.asarray(x, np.float32), np.asarray(w_qkv, np.float32),
                         np.asarray(w_proj, np.float32))
    res = run_bass_kernel_spmd(nc, in_maps, list(range(N_CORES)), trace=TRACE)
    _CACHE["last_result"] = res

    y = np.zeros((B, T, C), dtype=np.float64)
    for c in range(N_CORES):
        b = c // 4
        y[b] += res.results[c]["y"].astype(np.float64)
    return y.astype(np.float32)
